# revision 2
# baseline (speedup 1.0000x reference)
"""GCN encoder (2x GCNConv + linear projection, relu) on 8 Trainium2 cores, v2.

Key math identity: the GCNConv linear transform commutes with the
aggregation, so each layer aggregates RAW (pre-scaled) features and applies
the weight matrix once per destination window:

    agg[d]  = sum_{e: dst=d} xs[src_e] + xs[d]       (xs = dinv * x, host-scaled)
    h1[d]   = relu(dinv[d] * (agg[d] @ W1.T) + b1)
    hs1[d]  = dinv[d] * h1[d]                        (layer-2 gather table)
    agg2[d] = sum hs1[src_e] + hs1[d]
    h2[d]   = relu(dinv[d] * (agg2[d] @ W2.T) + b2)
    out[d]  = relu(h2[d] @ Wp.T + bp)

This removes the per-layer full-table transform of v1 entirely: layer 1
gathers directly from a host-provided bf16 table, so gather DMA starts
immediately.  Device mapping per core:
  - nodes sharded by contiguous range (6250/core, padded to 6272 = 49
    windows of 128)
  - edges partitioned by dst owner, grouped by (dst window, src piece)
  - table rows laid out piece-major (piece0 = all cores' windows 0..23,
    piece1 = windows 24..48) so the layer boundary AllGather splits into
    two pipelined collectives and idx lists stay int16
  - gather: gpsimd dma_gather, one call per (7-window group, piece)
  - segment-sum: per-128-edge selection matmul into a [feat x dst] PSUM
    tile (transposed orientation; selection matrices built 16 chunks per
    DVE op); self-loop added as a per-window identity matmul
  - biases folded into rank-1 matmuls (sqrt(deg) x b) inside PSUM groups
"""

import sys
import numpy as np

for _p in ("/opt/trn_rl_repo",):
    if _p not in sys.path:
        sys.path.append(_p)

import concourse.bacc as bacc
import concourse.tile as tile
from concourse import bass, mybir, bass_utils

F32 = mybir.dt.float32
BF16 = mybir.dt.bfloat16
I16 = mybir.dt.int16
AF = mybir.ActivationFunctionType
ALU = mybir.AluOpType
NP_BF16 = mybir.dt.np(BF16)


class Cfg:
    def __init__(self):
        self.N, self.E, self.CORES = 50000, 800000, 8
        self.C, self.OUT_C = 128, 64
        self.S = self.N // self.CORES                  # 6250 real nodes/shard
        self.SP = -(-self.S // 128) * 128              # 6272 padded
        self.NW = self.SP // 128                       # 49 windows
        self.W0 = 18                                   # windows in piece 0
        self.R0c = self.W0 * 128                       # 3072 piece-0 rows/core
        self.R1c = (self.NW - self.W0) * 128           # 3200 piece-1 rows/core
        self.R0 = self.R0c * self.CORES                # 24576
        self.R1 = self.R1c * self.CORES                # 25600
        assert self.R0 < 32768 and self.R1 < 32768
        self.GW = 7                                    # windows per gather grp
        self.NGRP = -(-self.NW // self.GW)             # 7 groups
        self.K = 16                                    # chunks per sel build
        self.WG = 6                                    # windows per table write
        self.SUB = 2048                                # max idx per gather call


CFG = Cfg()


def _wrap16(a):
    """[L] -> [128, L/16] int16 idx layout for dma_gather (16-wrap, 8x repl)."""
    assert a.size % 16 == 0 and a.size > 0
    w = a.reshape(-1, 16).T.astype(np.int16)
    return np.ascontiguousarray(np.tile(w, (8, 1)))


def _host_prep(cfg, x, edge_index):
    """Build per-core device inputs + the compile-time schedule."""
    N, C, S, SP, NW, CORES = cfg.N, cfg.C, cfg.S, cfg.SP, cfg.NW, cfg.CORES
    W0, R0c, R1c = cfg.W0, cfg.R0c, cfg.R1c

    src = np.asarray(edge_index[0]).astype(np.int64)
    dst = np.asarray(edge_index[1]).astype(np.int64)
    deg = (np.bincount(dst, minlength=N) + 1).astype(np.float64)
    dinv = (1.0 / np.sqrt(deg)).astype(np.float32)

    nodes = np.arange(N, dtype=np.int64)
    ncore = nodes // S
    nloc = nodes - ncore * S
    nwin = nloc // 128
    nrow = nloc % 128
    nhalf = nwin >= W0                                  # piece id per node
    nrid = np.where(nhalf,
                    ncore * R1c + (nwin - W0) * 128 + nrow,
                    ncore * R0c + nwin * 128 + nrow)    # row in piece region

    # shared gather tables (pre-scaled features, piece-major layout)
    xs = np.asarray(x, np.float32) * dinv[:, None]
    xsa = np.zeros((cfg.R0, C), np.float32)
    xsb = np.zeros((cfg.R1, C), np.float32)
    xsa[nrid[~nhalf]] = xs[~nhalf]
    xsb[nrid[nhalf]] = xs[nhalf]

    # edge partitioning: by dst owner, then (dst window, src piece)
    owner = dst // S
    dl = dst - owner * S
    dwin = dl // 128
    drel = (dl % 128).astype(np.float32)
    eh = nhalf[src].astype(np.int64)
    erow = nrid[src]

    key = (owner * NW + dwin) * 2 + eh
    counts = np.bincount(key, minlength=CORES * NW * 2).reshape(CORES, NW, 2)
    maxc = counts.max(axis=0)                           # [NW, 2]
    cap = -(-maxc // 128)                               # chunks per (w, h)

    per_core = []
    for c in range(CORES):
        m = owner == c
        cw, cr, cs, ch = dwin[m], drel[m], erow[m], eh[m]
        # sort by (window, half, src) — ascending src improves HBM locality
        order = np.lexsort((cs, ch, cw))
        cw, cr, cs, ch = cw[order], cr[order], cs[order], ch[order]
        k = cw * 2 + ch
        ia_parts, ib_parts, rel_parts = [], [], []
        for wi in range(NW):
            for half in (0, 1):
                cwh = int(cap[wi, half])
                lo = np.searchsorted(k, wi * 2 + half, "left")
                hi = np.searchsorted(k, wi * 2 + half, "right")
                n = hi - lo
                assert n <= cwh * 128
                iv = np.concatenate(
                    [cs[lo:hi], np.zeros(cwh * 128 - n, np.int64)])
                rv = np.concatenate(
                    [cr[lo:hi], np.full(cwh * 128 - n, -1.0, np.float32)])
                (ib_parts if half else ia_parts).append(iv)
                rel_parts.append(rv)
        # gather streams grouped by (group, half): group g = windows
        # GW*g .. GW*g+GW-1; within a group the windows' segments concat
        idxa = np.concatenate(
            [ia_parts[w] for g in range(cfg.NGRP)
             for w in range(g * cfg.GW, min((g + 1) * cfg.GW, NW))])
        idxb = np.concatenate(
            [ib_parts[w] for g in range(cfg.NGRP)
             for w in range(g * cfg.GW, min((g + 1) * cfg.GW, NW))])
        rel_all = np.concatenate(rel_parts).astype(np.float32)
        relT = np.ascontiguousarray(
            rel_all.reshape(-1, 128).T).astype(NP_BF16)

        # local per-core tensors
        lid = c * S + np.arange(S)
        dinv_loc = np.ones(SP, np.float32)
        dinv_loc[:S] = dinv[lid]
        deg_loc = np.ones(SP, np.float32)
        deg_loc[:S] = deg[lid].astype(np.float32)
        xlr = np.zeros((SP, C), np.float32)
        xlr[:S] = xs[lid]
        xlr = np.ascontiguousarray(
            xlr.reshape(NW, 128, C).transpose(1, 0, 2).reshape(128, NW * C))
        dinvl = np.ascontiguousarray(dinv_loc.reshape(NW, 128).T)
        dinvrow = np.ascontiguousarray(
            np.tile(dinv_loc[None, :], (128, 1)))
        sdeg = np.sqrt(deg_loc)[None, :]

        per_core.append(dict(
            idxa=_wrap16(idxa), idxb=_wrap16(idxb), rel=relT,
            xlr=xlr.astype(NP_BF16), dinvl=dinvl, dinvrow=dinvrow,
            sdeg=np.ascontiguousarray(sdeg).astype(NP_BF16)))

    sched = dict(cap=[[int(cap[w, h]) for h in (0, 1)] for w in range(NW)])
    shared = dict(xsa=xsa.astype(NP_BF16), xsb=xsb.astype(NP_BF16))
    return sched, shared, per_core


def _build_nc(cfg, sched):
    C, OUT_C, SP, NW = cfg.C, cfg.OUT_C, cfg.SP, cfg.NW
    W0, GW, NGRP, K, WG = cfg.W0, cfg.GW, cfg.NGRP, cfg.K, cfg.WG
    cap = sched["cap"]
    nchunk = sum(cap[w][h] for w in range(NW) for h in (0, 1))
    # per (group, half): idx count and per-window block base offsets
    glen = {}      # (g, h) -> idx count
    gbase = {}     # (w, h) -> block offset within its group tile
    goff = {0: 0, 1: 0}   # running idx column offsets per half stream
    gcoloff = {}   # (g, h) -> column offset (x16) into the idx stream
    for g in range(NGRP):
        ws = range(g * GW, min((g + 1) * GW, NW))
        for h in (0, 1):
            n = 0
            for w in ws:
                gbase[(w, h)] = n // 128
                n += cap[w][h] * 128
            glen[(g, h)] = n
            gcoloff[(g, h)] = goff[h] // 16
            goff[h] += n
    la16 = goff[0] // 16
    lb16 = goff[1] // 16
    gmaxblk = max(n // 128 for n in glen.values())

    nc = bacc.Bacc("TRN2", target_bir_lowering=False, debug=False,
                   enable_asserts=False, num_devices=cfg.CORES,
                   num_swdge_queues=4)

    def inp(name, shape, dt=F32):
        return nc.dram_tensor(name, shape, dt, kind="ExternalInput").ap()

    xsa_d = inp("xsa", [cfg.R0, C], BF16)
    xsb_d = inp("xsb", [cfg.R1, C], BF16)
    xlr_d = inp("xlr", [128, NW * C], BF16)
    w1t_d = inp("w1t", [C, C], BF16)
    w2t_d = inp("w2t", [C, C], BF16)
    wpt_d = inp("wpt", [C, OUT_C], BF16)
    b1r_d = inp("b1r", [1, C], BF16)
    b2r_d = inp("b2r", [1, C], BF16)
    bpr_d = inp("bpr", [1, OUT_C], BF16)
    ones1_d = inp("ones1", [1, 128], BF16)
    sdeg_d = inp("sdeg", [1, SP], BF16)
    dinvl_d = inp("dinvl", [128, NW])
    dinvrow_d = inp("dinvrow", [128, SP])
    identb_d = inp("identb", [128, 128], BF16)
    iotak_d = inp("iotak", [128, K * 128], BF16)
    idxa_d = inp("idxa", [128, max(la16, 16)], I16)
    idxb_d = inp("idxb", [128, max(lb16, 16)], I16)
    rel_d = inp("rel", [128, nchunk], BF16)
    out_d = nc.dram_tensor("out", [SP, OUT_C], F32, kind="ExternalOutput").ap()

    g2loc = nc.dram_tensor("g2loc", [SP, C], BF16, kind="Internal").ap()
    g2a = nc.dram_tensor("g2a", [cfg.R0, C], BF16, kind="Internal",
                         addr_space="Shared").ap()
    g2b = nc.dram_tensor("g2b", [cfg.R1, C], BF16, kind="Internal",
                         addr_space="Shared").ap()

    from contextlib import ExitStack
    with tile.TileContext(nc) as tc, ExitStack() as ctx:
        cp = ctx.enter_context(tc.tile_pool(name="consts", bufs=1))
        msgp = ctx.enter_context(tc.tile_pool(name="msg", bufs=6))
        selp = ctx.enter_context(tc.tile_pool(name="sel", bufs=3))
        epool = ctx.enter_context(tc.tile_pool(name="epi", bufs=4))
        ppT = ctx.enter_context(tc.tile_pool(name="ppT", bufs=2, space="PSUM"))
        ppW = ctx.enter_context(tc.tile_pool(name="ppW", bufs=2, space="PSUM"))
        ppP = ctx.enter_context(tc.tile_pool(name="ppP", bufs=2, space="PSUM"))

        def cload(name, ap, shape, dt=F32):
            t = cp.tile(shape, dt, tag=name)
            nc.sync.dma_start(t[:], ap[:])
            return t

        # idx lists first: the first gather's desc-gen waits only on these.
        # Split the load so the head columns (first gather pieces) land fast.
        def iload(name, ap, cols):
            t = cp.tile([128, cols], I16, tag=name)
            h = min(256, cols)
            nc.sync.dma_start(t[:, :h], ap[:, :h])
            if cols > h:
                nc.sync.dma_start(t[:, h:], ap[:, h:])
            return t

        idxa = iload("idxa", idxa_d, max(la16, 16))
        idxb = iload("idxb", idxb_d, max(lb16, 16))
        w1t = cload("w1t", w1t_d, [C, C], BF16)
        w2t = cload("w2t", w2t_d, [C, C], BF16)
        wpt = cload("wpt", wpt_d, [C, OUT_C], BF16)
        b1r = cload("b1r", b1r_d, [1, C], BF16)
        b2r = cload("b2r", b2r_d, [1, C], BF16)
        bpr = cload("bpr", bpr_d, [1, OUT_C], BF16)
        ones1 = cload("ones1", ones1_d, [1, 128], BF16)
        sdeg = cload("sdeg", sdeg_d, [1, SP], BF16)
        dinvl = cload("dinvl", dinvl_d, [128, NW])
        dinvrow = cload("dinvrow", dinvrow_d, [128, SP])
        identb = cload("identb", identb_d, [128, 128], BF16)
        iotak = cload("iotak", iotak_d, [128, K * 128], BF16)
        xlr = cload("xlr", xlr_d, [128, NW * C], BF16)
        rel = cload("rel", rel_d, [128, nchunk], BF16)

        hs1all = cp.tile([128, NW * C], BF16, tag="hs1all")
        oall = cp.tile([128, NW * OUT_C], F32, tag="oall")

        xlr3 = xlr[:].rearrange("p (w f) -> p w f", f=C)
        hs13 = hs1all[:].rearrange("p (w f) -> p w f", f=C)
        oall3 = oall[:].rearrange("p (w f) -> p w f", f=OUT_C)

        # batched selection-matrix builds: 16 chunks per DVE op
        selstate = {"buf": None, "base": -1}

        def get_sel(ci):
            b0 = (ci // K) * K
            if selstate["base"] != b0:
                kk = min(K, nchunk - b0)
                sb = selp.tile([128, K * 128], BF16, tag="sel")
                nc.vector.tensor_tensor(
                    out=sb[:, :kk * 128].rearrange("p (k f) -> p k f", f=128),
                    in0=rel[:, b0:b0 + kk].to_broadcast([128, kk, 128]),
                    in1=iotak[:, :kk * 128].rearrange(
                        "p (k f) -> p k f", f=128),
                    op=ALU.is_equal)
                selstate["buf"] = sb
                selstate["base"] = b0
            sb = selstate["buf"]
            j = ci - b0
            return sb[:, j * 128:(j + 1) * 128]

        qctr = [0]

        def emit_gather(tabA, tabB, g, lname):
            """dma_gather calls (<= SUB idx each) per (group, half).

            Each call gets its own msg tile so windows wait only on the
            sub-piece that carries their blocks.  Returns per half a list of
            (first_block, nblk, tile)."""
            out = {}
            for h, tab, isl in ((0, tabA, idxa), (1, tabB, idxb)):
                n = glen[(g, h)]
                pieces = []
                co = gcoloff[(g, h)]
                for s0 in range(0, n, cfg.SUB):
                    ns = min(cfg.SUB, n - s0)
                    nb = ns // 128
                    msg = msgp.tile([128, cfg.SUB // 128, C], BF16,
                                    tag=f"msg{h}")
                    nc.gpsimd.dma_gather(
                        msg[:, :nb, :], tab,
                        isl[:, co + s0 // 16:co + (s0 + ns) // 16], ns, ns,
                        elem_size=C, single_packet=False,
                        queue_num=qctr[0] % 4)
                    qctr[0] += 1
                    pieces.append((s0 // 128, nb, msg))
                out[h] = pieces
            return out

        def msg_block(pieces, blk):
            for b0, nb, msg in pieces:
                if b0 <= blk < b0 + nb:
                    return msg[:, blk - b0, :]
            raise AssertionError(blk)

        def layer(lname, tabA, tabB, self3, ci0):
            ci = ci0
            for g in range(NGRP):
                msgs = emit_gather(tabA, tabB, g, lname)
                for w in range(g * GW, min((g + 1) * GW, NW)):
                    psT = ppT.tile([128, 128], F32, tag="psT")
                    first = True
                    for h in (0, 1):
                        for j in range(cap[w][h]):
                            sel = get_sel(ci)
                            ci += 1
                            nc.tensor.matmul(
                                psT[:],
                                lhsT=msg_block(msgs[h], gbase[(w, h)] + j),
                                rhs=sel, start=first, stop=False)
                            first = False
                    nc.tensor.matmul(psT[:], lhsT=self3[:, w, :], rhs=identb[:],
                                     start=first, stop=True)
                    aggT = epool.tile([128, 128], BF16, tag="aggT")
                    nc.vector.tensor_copy(aggT[:], psT[:])
                    wsl = slice(w * 128, (w + 1) * 128)
                    if lname == "l1":
                        h1p = ppW.tile([128, C], F32, tag="h1p")
                        nc.tensor.matmul(h1p[:], lhsT=aggT[:], rhs=w1t[:],
                                         start=True, stop=False)
                        nc.tensor.matmul(h1p[:], lhsT=sdeg[:, wsl], rhs=b1r[:],
                                         start=False, stop=True)
                        t1 = epool.tile([128, C], F32, tag="t1")
                        nc.scalar.activation(t1[:], h1p[:], AF.Relu,
                                             scale=dinvl[:, w:w + 1])
                        nc.scalar.activation(hs13[:, w, :], t1[:], AF.Identity,
                                             scale=dinvl[:, w:w + 1])
                        # table write + collectives at write-group boundaries
                        if w % WG == WG - 1 or w == NW - 1:
                            wlo = (w // WG) * WG
                            nc.sync.dma_start(
                                g2loc[wlo * 128:(w + 1) * 128, :].rearrange(
                                    "(j p) f -> p j f", p=128),
                                hs13[:, wlo:w + 1, :])
                        if w == W0 - 1:
                            nc.gpsimd.collective_compute(
                                "AllGather", ALU.bypass,
                                replica_groups=[list(range(cfg.CORES))],
                                ins=[g2loc[0:cfg.R0c, :]], outs=[g2a[:]])
                        if w == NW - 1:
                            nc.gpsimd.collective_compute(
                                "AllGather", ALU.bypass,
                                replica_groups=[list(range(cfg.CORES))],
                                ins=[g2loc[cfg.R0c:SP, :]], outs=[g2b[:]])
                    else:
                        h2pT = ppW.tile([128, 128], F32, tag="h2pT")
                        nc.tensor.matmul(h2pT[:], lhsT=w2t[:], rhs=aggT[:],
                                         start=True, stop=False)
                        nc.tensor.matmul(h2pT[:], lhsT=b2r[:],
                                         rhs=sdeg[:, wsl],
                                         start=False, stop=True)
                        t2 = epool.tile([128, 128], F32, tag="t2")
                        nc.vector.tensor_tensor(out=t2[:], in0=h2pT[:],
                                                in1=dinvrow[:, wsl],
                                                op=ALU.mult)
                        h2T = epool.tile([128, 128], BF16, tag="h2T")
                        nc.scalar.activation(h2T[:], t2[:], AF.Relu)
                        pp = ppP.tile([128, OUT_C], F32, tag="pp")
                        nc.tensor.matmul(pp[:], lhsT=h2T[:], rhs=wpt[:],
                                         start=True, stop=False)
                        nc.tensor.matmul(pp[:], lhsT=ones1[:], rhs=bpr[:],
                                         start=False, stop=True)
                        nc.scalar.activation(oall3[:, w, :], pp[:], AF.Relu)
                        if w % WG == WG - 1 or w == NW - 1:
                            wlo = (w // WG) * WG
                            nc.sync.dma_start(
                                out_d[wlo * 128:(w + 1) * 128, :].rearrange(
                                    "(j p) f -> p j f", p=128),
                                oall3[:, wlo:w + 1, :])
            return ci

        layer("l1", xsa_d, xsb_d, xlr3, 0)
        selstate["base"] = -1
        layer("l2", g2a, g2b, hs13, 0)

    nc.compile()
    return nc


def _make_in_maps(cfg, sched, shared, per_core, W1, b1, W2, b2, Wp, bp):
    K = cfg.K
    w1t = np.ascontiguousarray(np.asarray(W1, np.float32).T).astype(NP_BF16)
    w2t = np.ascontiguousarray(np.asarray(W2, np.float32).T).astype(NP_BF16)
    wpt = np.ascontiguousarray(np.asarray(Wp, np.float32).T).astype(NP_BF16)
    base = dict(
        xsa=shared["xsa"], xsb=shared["xsb"], w1t=w1t, w2t=w2t, wpt=wpt,
        b1r=np.asarray(b1, np.float32)[None, :].astype(NP_BF16),
        b2r=np.asarray(b2, np.float32)[None, :].astype(NP_BF16),
        bpr=np.asarray(bp, np.float32)[None, :].astype(NP_BF16),
        ones1=np.ones((1, 128), NP_BF16),
        identb=np.eye(128, dtype=np.float32).astype(NP_BF16),
        iotak=np.tile(np.arange(128, dtype=np.float32)[None, :],
                      (128, K)).astype(NP_BF16))
    in_maps = []
    for c in range(cfg.CORES):
        m = dict(base)
        m.update(per_core[c])
        in_maps.append(m)
    return in_maps


def _run(inputs, cfg=None, trace=False, tmpdir=None, verbose=True):
    import time
    t0 = time.time()

    def _log(msg):
        if verbose:
            print(f"[kernel {time.time()-t0:7.1f}s] {msg}", flush=True)
    cfg = cfg or CFG
    sched, shared, per_core = _host_prep(cfg, inputs["x"],
                                         inputs["edge_index"])
    _log("host prep done")
    nc = _build_nc(cfg, sched)
    _log("build+compile done")
    in_maps = _make_in_maps(cfg, sched, shared, per_core,
                            inputs["W1"], inputs["b1"], inputs["W2"],
                            inputs["b2"], inputs["Wp"], inputs["bp"])
    _log("in_maps done")
    core_ids = list(range(cfg.CORES))
    if trace:
        bass_utils.run_bass_kernel_spmd(nc, in_maps, core_ids=core_ids,
                                        trace=False)
        _log("warmup run done")
    res = bass_utils.run_bass_kernel_spmd(nc, in_maps, core_ids=core_ids,
                                          trace=trace, tmpdir=tmpdir)
    _log("run done")
    out = np.empty((cfg.N, cfg.OUT_C), np.float32)
    for c in range(cfg.CORES):
        out[c * cfg.S:(c + 1) * cfg.S] = res.results[c]["out"][:cfg.S]
    return out, res


def kernel(**inputs):
    out, _ = _run(inputs)
    return out
